# revision 1
# baseline (speedup 1.0000x reference)
"""Fused QKV projection (nn.Linear premix) on 8 Trainium2 NeuronCores.

qkv = x @ W_qkv^T ; split into per-head q,k,v of shape [B,H,S,DK].

Sharding (tensor-parallel, per spec hint): the 3E=6144 output dim of
W_qkv is head-sharded across 8 cores.  Core c owns q-heads {2c,2c+1},
k-heads {2c,2c+1}, v-heads {2c,2c+1} -> 768 rows of W.  x is replicated.

Per-core GEMM: [16384 x 2048] @ [2048 x 768].

Device kernel design:
  - Inputs are pre-cast to bf16 and pre-transposed on the host so every
    DMA is a natural, contiguous load:
      xt [16, 128, 16384]  : xt[kt, p, m] = x[m, kt*128+p]   (x^T tiles)
      wt [128, 16, 768]    : wt[p, kt, f] = W_c[f, kt*128+p] (W_c^T tiles)
  - W_c^T (3 MB bf16) stays SBUF-resident for the whole kernel.
  - Loop over 512-token super-tiles: one 2 MB DMA brings in x^T, then
    4x token-subtiles of 128: two PSUM accumulation chains (N=384) of
    16 matmuls each (contraction E=2048 in 16 steps of 128).
  - PSUM (fp32) drained by VectorE into SBUF, then 6 contiguous 64 KB
    DMAs write one [128 tokens x 128 dk] block per owned head-slice.
  - All matmuls are bf16 x bf16 -> fp32 PSUM (1 cycle/row on TensorE).
"""

import numpy as np
import ml_dtypes

B, S, E, H, DK = 4, 4096, 2048, 16, 128
M = B * S              # 16384 tokens
NCORES = 8
FPC = 3 * E // NCORES  # 768 output features per core (6 head-slices)
KT = E // 128          # 16 contraction subtiles
TOK_SUPER = 512
N_SUPER = M // TOK_SUPER
NHALF = FPC // 2       # 384: two PSUM chains per token-subtile

_cache = {}


def _build_program():
    import concourse.bass as bass
    import concourse.bacc as bacc
    import concourse.mybir as mybir
    from concourse import tile

    ts = bass.ts
    nc = bacc.Bacc("TRN2", target_bir_lowering=False, debug=False,
                   num_devices=NCORES)
    xt = nc.dram_tensor("xt", [KT, 128, M], mybir.dt.bfloat16,
                        kind="ExternalInput")
    wt = nc.dram_tensor("wt", [128, KT, FPC], mybir.dt.bfloat16,
                        kind="ExternalInput")
    out = nc.dram_tensor("out", [6, M, DK], mybir.dt.float32,
                         kind="ExternalOutput")

    KC = 4                 # kt chunks per super-tile (finer DMA/compute dep)
    KPC = KT // KC         # 4 kt per chunk
    with tile.TileContext(nc) as tc:
        with tc.tile_pool(name="wpool", bufs=1) as wpool, \
             tc.tile_pool(name="xpool", bufs=3) as xpool, \
             tc.tile_pool(name="opool", bufs=6) as opool, \
             tc.tile_pool(name="pspool", bufs=3, space="PSUM") as pspool:
            # W as KC independent tiles on the Scalar HWDGE queue: issue
            # parallelizes with the x loads on Sync, and the first
            # accumulation chain only waits for its own 768 KB slab
            # (Tile deps are per-tile).
            wsb = []
            for kc in range(KC):
                wc = wpool.tile([128, KPC, FPC], mybir.dt.bfloat16,
                                tag=f"w{kc}")
                nc.scalar.dma_start(wc[:], wt[:, ts(kc, KPC), :])
                wsb.append(wc)
            for st in range(N_SUPER):
                xsb = []
                for kc in range(KC):
                    xc = xpool.tile([128, KPC, TOK_SUPER], mybir.dt.bfloat16,
                                    tag=f"x{kc}")
                    nc.sync.dma_start(
                        xc[:],
                        xt[ts(kc, KPC), :, ts(st, TOK_SUPER)]
                        .rearrange("k p m -> p k m"))
                    xsb.append(xc)
                for sub in range(TOK_SUPER // 128):
                    ps0 = pspool.tile([128, NHALF], mybir.dt.float32)
                    ps1 = pspool.tile([128, NHALF], mybir.dt.float32)
                    for kt in range(KT):
                        lhsT = xsb[kt // KPC][:, kt % KPC, ts(sub, 128)]
                        wv = wsb[kt // KPC][:, kt % KPC, :]
                        nc.tensor.matmul(ps0[:], lhsT,
                                         wv[:, 0:NHALF],
                                         start=(kt == 0), stop=(kt == KT - 1))
                        nc.tensor.matmul(ps1[:], lhsT,
                                         wv[:, NHALF:FPC],
                                         start=(kt == 0), stop=(kt == KT - 1))
                    osb = opool.tile([128, FPC], mybir.dt.float32)
                    nc.vector.tensor_copy(osb[:, 0:NHALF], ps0[:])
                    nc.vector.tensor_copy(osb[:, NHALF:FPC], ps1[:])
                    m0 = st * TOK_SUPER + sub * 128
                    for j in range(6):
                        nc.sync.dma_start(out[j, m0:m0 + 128, :],
                                          osb[:, ts(j, DK)])
    nc.compile()
    return nc


def _host_inputs(x, W_qkv):
    bf16 = ml_dtypes.bfloat16
    xf = np.ascontiguousarray(np.asarray(x, dtype=np.float32).reshape(M, E))
    xt = np.ascontiguousarray(
        xf.reshape(M, KT, 128).astype(bf16).transpose(1, 2, 0))
    W = np.asarray(W_qkv, dtype=np.float32)
    in_maps = []
    for c in range(NCORES):
        rows = np.concatenate([W[o + 256 * c: o + 256 * c + 256]
                               for o in (0, E, 2 * E)])
        wt_c = np.ascontiguousarray(
            rows.reshape(FPC, KT, 128).astype(bf16).transpose(2, 1, 0))
        in_maps.append({"xt": xt, "wt": wt_c})
    return in_maps


def kernel(x, W_qkv):
    from concourse.bass_utils import run_bass_kernel_spmd

    if "nc" not in _cache:
        _cache["nc"] = _build_program()
    nc = _cache["nc"]

    in_maps = _host_inputs(x, W_qkv)
    res = run_bass_kernel_spmd(nc, in_maps, core_ids=list(range(NCORES)))
    kernel._last_results = res

    q = np.empty((B, H, S, DK), np.float32)
    k = np.empty_like(q)
    v = np.empty_like(q)
    for c in range(NCORES):
        o = res.results[c]["out"]          # [6, M, DK]
        for j in range(2):
            q[:, 2 * c + j] = o[j].reshape(B, S, DK)
            k[:, 2 * c + j] = o[2 + j].reshape(B, S, DK)
            v[:, 2 * c + j] = o[4 + j].reshape(B, S, DK)
    return q, k, v



# revision 2
# speedup vs baseline: 1.0087x; 1.0087x over previous
"""Fused QKV projection (nn.Linear premix) on 8 Trainium2 NeuronCores.

qkv = x @ W_qkv^T ; split into per-head q,k,v of shape [B,H,S,DK].

Sharding (tensor-parallel, per spec hint): the 3E=6144 output dim of
W_qkv is head-sharded across 8 cores.  Core c owns q-heads {2c,2c+1},
k-heads {2c,2c+1}, v-heads {2c,2c+1} -> 768 rows of W.  x is replicated.

Per-core GEMM: [16384 x 2048] @ [2048 x 768] in bf16 (TensorE peak
78.6 TF/s -> ~656 us floor; measured stream rate 163 ns per N=384 MM).

Device kernel design (v2 — head/tail optimized):
  - Host pre-layouts so every DMA line is long and contiguous:
      xh  [32, 128, 16, 512] : xh[st,p,kt,m] = x[st*512+m, kt*128+p]
          -> one 2 MB DMA per 512-token super-tile with 16 KB lines
             (supertile 0 split into 4x512 KB chunks, 4 KB lines, so
             the first matmul chain can start ASAP).
      wt  [128, 16, 768]     : wt[p,kt,f] = W_c[f, kt*128+p]
  - Warm-up burst: ~11 N=512 matmuls on a zeroed SBUF tile issued
    right after the preamble barrier.  They depend on no DMA, keep the
    PE busy through the initial HBM loads, and flip the HAM clock gate
    to 8/8 (2.4 GHz) before the first real matmul.
  - Per 128-token subtile: two PSUM accumulation chains (N=384) of 16
    bf16 matmuls; VectorE drains PSUM (fp32) into a bf16 [128,768]
    SBUF tile; ONE contiguous 196 KB DMA per subtile writes
    out[m0:m0+128, :] (bf16, 1.5 KB rows).
  - Queue split: x-in on the sync HWDGE ring, W-in + out on the scalar
    HWDGE ring, so input and output streams never serialize on one NX
    and the tail drain is a single short DMA.
"""

import numpy as np
import ml_dtypes

B, S, E, H, DK = 4, 4096, 2048, 16, 128
M = B * S              # 16384 tokens
NCORES = 8
FPC = 3 * E // NCORES  # 768 output features per core (6 head-slices)
KT = E // 128          # 16 contraction subtiles
TOK_SUPER = 512
N_SUPER = M // TOK_SUPER
NHALF = FPC // 2       # 384: two PSUM chains per token-subtile

_cache = {}


def _build_program():
    import concourse.bass as bass
    import concourse.bacc as bacc
    import concourse.mybir as mybir
    from concourse import tile

    ts = bass.ts
    nc = bacc.Bacc("TRN2", target_bir_lowering=False, debug=False,
                   num_devices=NCORES)
    xh = nc.dram_tensor("xh", [N_SUPER, 128, KT, TOK_SUPER],
                        mybir.dt.bfloat16, kind="ExternalInput")
    wt = nc.dram_tensor("wt", [128, KT, FPC], mybir.dt.bfloat16,
                        kind="ExternalInput")
    out = nc.dram_tensor("out", [M, FPC], mybir.dt.bfloat16,
                         kind="ExternalOutput")

    KC = 4                 # kt chunks for W and for supertile 0's x
    KPC = KT // KC         # 4 kt per chunk
    with tile.TileContext(nc) as tc:
        with tc.tile_pool(name="zpool", bufs=1) as zpool, \
             tc.tile_pool(name="wpool", bufs=1) as wpool, \
             tc.tile_pool(name="x0pool", bufs=1) as x0pool, \
             tc.tile_pool(name="xpool", bufs=3) as xpool, \
             tc.tile_pool(name="opool", bufs=4) as opool, \
             tc.tile_pool(name="wmps", bufs=1, space="PSUM") as wmps, \
             tc.tile_pool(name="pspool", bufs=3, space="PSUM") as pspool:
            # ---- warm-up: PE busy from ~t0 so HAM un-throttles before
            # the first real matmul; zero source, dedicated PSUM bank.
            zt = zpool.tile([128, 512], mybir.dt.bfloat16, tag="z")
            nc.vector.memset(zt[:], 0.0)
            pw = wmps.tile([128, 512], mybir.dt.float32, tag="pw")
            for _ in range(11):
                nc.tensor.matmul(pw[:], zt[:, 0:128], zt[:],
                                 start=True, stop=True)

            # ---- W: 4 chunks on the scalar ring (6 KB lines)
            wsb = []
            for kc in range(KC):
                wc = wpool.tile([128, KPC, FPC], mybir.dt.bfloat16,
                                tag=f"w{kc}")
                nc.scalar.dma_start(wc[:], wt[:, ts(kc, KPC), :])
                wsb.append(wc)

            # ---- x supertile 0: 4 chunks on sync (4 KB lines) so the
            # first chain starts as soon as chunk 0 + W chunk 0 land.
            x0 = []
            for kc in range(KC):
                xc = x0pool.tile([128, KPC, TOK_SUPER], mybir.dt.bfloat16,
                                 tag=f"x0{kc}")
                nc.sync.dma_start(xc[:], xh[0, :, ts(kc, KPC), :])
                x0.append(xc)

            def compute_supertile(st, xslice):
                # xslice(kt) -> [128, 128] lhsT for token subtile sub
                for sub in range(TOK_SUPER // 128):
                    ps0 = pspool.tile([128, NHALF], mybir.dt.float32)
                    ps1 = pspool.tile([128, NHALF], mybir.dt.float32)
                    for kt in range(KT):
                        lhsT = xslice(kt, sub)
                        wv = wsb[kt // KPC][:, kt % KPC, :]
                        nc.tensor.matmul(ps0[:], lhsT,
                                         wv[:, 0:NHALF],
                                         start=(kt == 0), stop=(kt == KT - 1))
                        nc.tensor.matmul(ps1[:], lhsT,
                                         wv[:, NHALF:FPC],
                                         start=(kt == 0), stop=(kt == KT - 1))
                    osb = opool.tile([128, FPC], mybir.dt.bfloat16)
                    nc.vector.tensor_copy(osb[:, 0:NHALF], ps0[:])
                    nc.vector.tensor_copy(osb[:, NHALF:FPC], ps1[:])
                    m0 = st * TOK_SUPER + sub * 128
                    nc.scalar.dma_start(out[m0:m0 + 128, :], osb[:])

            compute_supertile(
                0, lambda kt, sub: x0[kt // KPC][:, kt % KPC, ts(sub, 128)])
            for st in range(1, N_SUPER):
                xs = xpool.tile([128, KT, TOK_SUPER], mybir.dt.bfloat16,
                                tag="xs")
                nc.sync.dma_start(xs[:], xh[st])
                compute_supertile(
                    st, lambda kt, sub, xs=xs: xs[:, kt, ts(sub, 128)])
    nc.compile()
    return nc


def _host_inputs(x, W_qkv):
    bf16 = ml_dtypes.bfloat16
    xf = np.asarray(x, dtype=np.float32).reshape(M, E).astype(bf16)
    # xh[st, p, kt, m] = x[st*512+m, kt*128+p]
    xh = np.ascontiguousarray(
        xf.reshape(N_SUPER, TOK_SUPER, KT, 128).transpose(0, 3, 2, 1))
    W = np.asarray(W_qkv, dtype=np.float32)
    in_maps = []
    for c in range(NCORES):
        rows = np.concatenate([W[o + 256 * c: o + 256 * c + 256]
                               for o in (0, E, 2 * E)])
        wt_c = np.ascontiguousarray(
            rows.reshape(FPC, KT, 128).astype(bf16).transpose(2, 1, 0))
        in_maps.append({"xh": xh, "wt": wt_c})
    return in_maps


def kernel(x, W_qkv):
    from concourse.bass_utils import run_bass_kernel_spmd

    if "nc" not in _cache:
        _cache["nc"] = _build_program()
    nc = _cache["nc"]

    in_maps = _host_inputs(x, W_qkv)
    res = run_bass_kernel_spmd(nc, in_maps, core_ids=list(range(NCORES)))
    kernel._last_results = res

    q = np.empty((B, H, S, DK), np.float32)
    k = np.empty_like(q)
    v = np.empty_like(q)
    for c in range(NCORES):
        o = res.results[c]["out"].reshape(B, S, 6, DK)   # bf16
        for j in range(2):
            q[:, 2 * c + j] = o[:, :, j].astype(np.float32)
            k[:, 2 * c + j] = o[:, :, 2 + j].astype(np.float32)
            v[:, 2 * c + j] = o[:, :, 4 + j].astype(np.float32)
    return q, k, v


# revision 3
# speedup vs baseline: 1.0136x; 1.0048x over previous
"""Fused QKV projection (nn.Linear premix) on 8 Trainium2 NeuronCores.

qkv = x @ W_qkv^T ; split into per-head q,k,v of shape [B,H,S,DK].

Sharding (tensor-parallel, per spec hint): the 3E=6144 output dim of
W_qkv is head-sharded across 8 cores.  Core c owns q-heads {2c,2c+1},
k-heads {2c,2c+1}, v-heads {2c,2c+1} -> 768 rows of W.  x is replicated.

Per-core GEMM: [16384 x 2048] @ [2048 x 768] in bf16 (TensorE peak
78.6 TF/s -> ~656 us floor).

Device kernel design (v3 — W-stationary, N=512):
  - W-stationary matmuls: lhsT = W^T block [128k, 128f], moving rhs =
    x^T [128k, 512 tokens] -> PSUM [128f, 512tok] fp32 = exactly one
    full PSUM bank.  3072 MMs total (vs 4096 for x-stationary N=384):
    same streamed columns, 25% fewer instruction overheads.
  - Host pre-layouts so every DMA line is long and contiguous:
      xh  [32, 128, 16, 512] : xh[st,p,kt,m] = x[st*512+m, kt*128+p]
          -> one 2 MB DMA per 512-token super-tile with 16 KB lines
             (supertile 0 split into 4x512 KB chunks, 4 KB lines).
      wt  [128, 16, 768]     : wt[p,kt,f] = W_c[f, kt*128+p]
  - Warm-up burst: ~11 N=512 matmuls on a zeroed SBUF tile right after
    the preamble barrier keep the PE busy through the initial HBM loads
    and flip the HAM clock gate to 8/8 before the first real matmul.
  - Head is data-bound on (W 3MB + x-st0 2MB): both go on the sync
    HWDGE ring, interleaved W-chunk/x-chunk, so W (which every chain
    needs in full) finishes as early as possible.
  - Output: per (supertile, f-block) chain, VectorE casts PSUM fp32 ->
    bf16 [128, 512] SBUF tile; one 128 KB DMA on the scalar ring writes
    out[f-block, token-slice] (1 KB rows).  In/out streams never share
    a descriptor ring.
"""

import numpy as np
import ml_dtypes

B, S, E, H, DK = 4, 4096, 2048, 16, 128
M = B * S              # 16384 tokens
NCORES = 8
FPC = 3 * E // NCORES  # 768 output features per core (6 head-slices)
FB = FPC // 128        # 6 feature blocks (head-slices)
KT = E // 128          # 16 contraction subtiles
TOK_SUPER = 512
N_SUPER = M // TOK_SUPER

_cache = {}


def _build_program():
    import concourse.bass as bass
    import concourse.bacc as bacc
    import concourse.mybir as mybir
    from concourse import tile

    ts = bass.ts
    nc = bacc.Bacc("TRN2", target_bir_lowering=False, debug=False,
                   num_devices=NCORES)
    xh = nc.dram_tensor("xh", [N_SUPER, 128, KT, TOK_SUPER],
                        mybir.dt.bfloat16, kind="ExternalInput")
    wt = nc.dram_tensor("wt", [128, KT, FPC], mybir.dt.bfloat16,
                        kind="ExternalInput")
    out = nc.dram_tensor("out", [FPC, M], mybir.dt.bfloat16,
                         kind="ExternalOutput")

    KC = 4                 # kt chunks for W and for supertile 0's x
    KPC = KT // KC         # 4 kt per chunk
    with tile.TileContext(nc) as tc:
        with tc.tile_pool(name="zpool", bufs=1) as zpool, \
             tc.tile_pool(name="wpool", bufs=1) as wpool, \
             tc.tile_pool(name="x0pool", bufs=1) as x0pool, \
             tc.tile_pool(name="xpool", bufs=3) as xpool, \
             tc.tile_pool(name="opool", bufs=4) as opool, \
             tc.tile_pool(name="wmps", bufs=1, space="PSUM") as wmps, \
             tc.tile_pool(name="pspool", bufs=3, space="PSUM") as pspool:
            # ---- warm-up: PE busy from ~t0 so HAM un-throttles before
            # the first real matmul; zero source, dedicated PSUM bank.
            zt = zpool.tile([128, 512], mybir.dt.bfloat16, tag="z")
            nc.vector.memset(zt[:], 0.0)
            pw = wmps.tile([128, 512], mybir.dt.float32, tag="pw")
            for _ in range(11):
                nc.tensor.matmul(pw[:], zt[:, 0:128], zt[:],
                                 start=True, stop=True)

            # ---- W and x-supertile-0 interleaved on the sync ring.
            # Every chain needs all 16 kt of W, so W's completion gates
            # the head; x st0 chunks ride along between W chunks.
            wsb = []
            x0 = []
            for kc in range(KC):
                wc = wpool.tile([128, KPC, FPC], mybir.dt.bfloat16,
                                tag=f"w{kc}")
                nc.sync.dma_start(wc[:], wt[:, ts(kc, KPC), :])
                wsb.append(wc)
                xc = x0pool.tile([128, KPC, TOK_SUPER], mybir.dt.bfloat16,
                                 tag=f"x0{kc}")
                nc.sync.dma_start(xc[:], xh[0, :, ts(kc, KPC), :])
                x0.append(xc)

            def compute_supertile(st, xslice):
                # xslice(kt) -> [128, 512] moving operand (x^T)
                for fb in range(FB):
                    ps = pspool.tile([128, TOK_SUPER], mybir.dt.float32)
                    for kt in range(KT):
                        wv = wsb[kt // KPC][:, kt % KPC, ts(fb, 128)]
                        nc.tensor.matmul(ps[:], wv, xslice(kt),
                                         start=(kt == 0), stop=(kt == KT - 1))
                    osb = opool.tile([128, TOK_SUPER], mybir.dt.bfloat16)
                    nc.vector.tensor_copy(osb[:], ps[:])
                    nc.scalar.dma_start(
                        out[ts(fb, 128), ts(st, TOK_SUPER)], osb[:])

            compute_supertile(
                0, lambda kt: x0[kt // KPC][:, kt % KPC, :])
            for st in range(1, N_SUPER):
                xs = xpool.tile([128, KT, TOK_SUPER], mybir.dt.bfloat16,
                                tag="xs")
                nc.sync.dma_start(xs[:], xh[st])
                compute_supertile(st, lambda kt, xs=xs: xs[:, kt, :])
    nc.compile()
    return nc


def _host_inputs(x, W_qkv):
    bf16 = ml_dtypes.bfloat16
    xf = np.asarray(x, dtype=np.float32).reshape(M, E).astype(bf16)
    # xh[st, p, kt, m] = x[st*512+m, kt*128+p]
    xh = np.ascontiguousarray(
        xf.reshape(N_SUPER, TOK_SUPER, KT, 128).transpose(0, 3, 2, 1))
    W = np.asarray(W_qkv, dtype=np.float32)
    in_maps = []
    for c in range(NCORES):
        rows = np.concatenate([W[o + 256 * c: o + 256 * c + 256]
                               for o in (0, E, 2 * E)])
        wt_c = np.ascontiguousarray(
            rows.reshape(FPC, KT, 128).astype(bf16).transpose(2, 1, 0))
        in_maps.append({"xh": xh, "wt": wt_c})
    return in_maps


def kernel(x, W_qkv):
    from concourse.bass_utils import run_bass_kernel_spmd

    if "nc" not in _cache:
        _cache["nc"] = _build_program()
    nc = _cache["nc"]

    in_maps = _host_inputs(x, W_qkv)
    res = run_bass_kernel_spmd(nc, in_maps, core_ids=list(range(NCORES)))
    kernel._last_results = res

    q = np.empty((B, H, S, DK), np.float32)
    k = np.empty_like(q)
    v = np.empty_like(q)
    for c in range(NCORES):
        o = res.results[c]["out"]                       # [768, 16384] bf16
        # arr[b, fb, s, dk] = o[fb*128+dk, b*4096+s]
        arr = np.ascontiguousarray(
            o.reshape(FB, 128, B, S).transpose(2, 0, 3, 1)).astype(np.float32)
        for j in range(2):
            q[:, 2 * c + j] = arr[:, j]
            k[:, 2 * c + j] = arr[:, 2 + j]
            v[:, 2 * c + j] = arr[:, 4 + j]
    return q, k, v


# revision 6
# speedup vs baseline: 1.0144x; 1.0008x over previous
"""Fused QKV projection (nn.Linear premix) on 8 Trainium2 NeuronCores.

qkv = x @ W_qkv^T ; split into per-head q,k,v of shape [B,H,S,DK].

Sharding (tensor-parallel, per spec hint): the 3E=6144 output dim of
W_qkv is head-sharded across 8 cores.  Core c owns q-heads {2c,2c+1},
k-heads {2c,2c+1}, v-heads {2c,2c+1} -> 768 rows of W.  x is replicated.

Per-core GEMM: [16384 x 2048] @ [2048 x 768] in bf16 (TensorE peak
78.6 TF/s -> ~656 us floor).

Device kernel design (v3 — W-stationary, N=512):
  - W-stationary matmuls: lhsT = W^T block [128k, 128f], moving rhs =
    x^T [128k, 512 tokens] -> PSUM [128f, 512tok] fp32 = exactly one
    full PSUM bank.  3072 MMs total (vs 4096 for x-stationary N=384):
    same streamed columns, 25% fewer instruction overheads.
  - Host pre-layouts so every DMA line is long and contiguous:
      xh  [32, 128, 16, 512] : xh[st,p,kt,m] = x[st*512+m, kt*128+p]
          -> one 2 MB DMA per 512-token super-tile with 16 KB lines
             (supertile 0 split into 4x512 KB chunks, 4 KB lines).
      wt  [128, 16, 768]     : wt[p,kt,f] = W_c[f, kt*128+p]
  - Warm-up burst: ~11 N=512 matmuls on a zeroed SBUF tile right after
    the preamble barrier keep the PE busy through the initial HBM loads
    and flip the HAM clock gate to 8/8 before the first real matmul.
  - Head is data-bound on (W 3MB + x-st0 2MB): both go on the sync
    HWDGE ring, interleaved W-chunk/x-chunk, so W (which every chain
    needs in full) finishes as early as possible.
  - Output: per (supertile, f-block) chain, VectorE casts PSUM fp32 ->
    bf16 [128, 512] SBUF tile; one 128 KB DMA on the scalar ring writes
    out[f-block, token-slice] (1 KB rows).  In/out streams never share
    a descriptor ring.
"""

import numpy as np
import ml_dtypes

B, S, E, H, DK = 4, 4096, 2048, 16, 128
M = B * S              # 16384 tokens
NCORES = 8
FPC = 3 * E // NCORES  # 768 output features per core (6 head-slices)
FB = FPC // 128        # 6 feature blocks (head-slices)
KT = E // 128          # 16 contraction subtiles
TOK_SUPER = 512
N_SUPER = M // TOK_SUPER

_cache = {}


def _build_program():
    import concourse.bass as bass
    import concourse.bacc as bacc
    import concourse.mybir as mybir
    from concourse import tile

    ts = bass.ts
    nc = bacc.Bacc("TRN2", target_bir_lowering=False, debug=False,
                   num_devices=NCORES)
    xh = nc.dram_tensor("xh", [N_SUPER, 128, KT, TOK_SUPER],
                        mybir.dt.bfloat16, kind="ExternalInput")
    wt = nc.dram_tensor("wt", [128, KT, FPC], mybir.dt.bfloat16,
                        kind="ExternalInput")
    out = nc.dram_tensor("out", [FPC, M], mybir.dt.bfloat16,
                         kind="ExternalOutput")

    KC = 8                 # kt chunks for W and for supertile 0's x
    KPC = KT // KC         # 2 kt per chunk
    with tile.TileContext(nc) as tc:
        with tc.tile_pool(name="zpool", bufs=1) as zpool, \
             tc.tile_pool(name="wpool", bufs=1) as wpool, \
             tc.tile_pool(name="x0pool", bufs=1) as x0pool, \
             tc.tile_pool(name="xpool", bufs=3) as xpool, \
             tc.tile_pool(name="opool", bufs=4) as opool, \
             tc.tile_pool(name="wmps", bufs=1, space="PSUM") as wmps, \
             tc.tile_pool(name="pspool", bufs=3, space="PSUM") as pspool:
            # ---- warm-up: PE busy from ~t0 so HAM un-throttles before
            # the first real matmul; zero source, dedicated PSUM bank.
            zt = zpool.tile([128, 512], mybir.dt.bfloat16, tag="z")
            nc.vector.memset(zt[:], 0.0)
            pw = wmps.tile([128, 512], mybir.dt.float32, tag="pw")
            for _ in range(8):
                nc.tensor.matmul(pw[:], zt[:, 0:128], zt[:],
                                 start=True, stop=True)

            # ---- W and x-supertile-0 interleaved on the sync ring.
            # Every chain needs all 16 kt of W, so W's completion gates
            # the head; x st0 chunks ride along between W chunks.
            wsb = []
            x0 = []
            for kc in range(KC):
                wc = wpool.tile([128, KPC, FPC], mybir.dt.bfloat16,
                                tag=f"w{kc}")
                nc.sync.dma_start(wc[:], wt[:, ts(kc, KPC), :])
                wsb.append(wc)
                xc = x0pool.tile([128, KPC, TOK_SUPER], mybir.dt.bfloat16,
                                 tag=f"x0{kc}")
                nc.sync.dma_start(xc[:], xh[0, :, ts(kc, KPC), :])
                x0.append(xc)

            def compute_supertile(st, xslice):
                # xslice(kt) -> [128, 512] moving operand (x^T)
                for fb in range(FB):
                    if st == N_SUPER - 1 and fb == FB - 1:
                        # final chain: two N=256 halves so the last
                        # PSUM drain + out-DMA receipt is half-size
                        # (shortens the serial kernel tail)
                        for h in range(2):
                            ps = pspool.tile([128, TOK_SUPER // 2],
                                             mybir.dt.float32)
                            for kt in range(KT):
                                wv = wsb[kt // KPC][:, kt % KPC, ts(fb, 128)]
                                nc.tensor.matmul(
                                    ps[:], wv,
                                    xslice(kt)[:, ts(h, TOK_SUPER // 2)],
                                    start=(kt == 0), stop=(kt == KT - 1))
                            osb = opool.tile([128, TOK_SUPER // 2],
                                             mybir.dt.bfloat16)
                            nc.vector.tensor_copy(osb[:], ps[:])
                            nc.scalar.dma_start(
                                out[ts(fb, 128),
                                    st * TOK_SUPER + h * (TOK_SUPER // 2):
                                    st * TOK_SUPER + (h + 1) * (TOK_SUPER // 2)],
                                osb[:])
                        continue
                    ps = pspool.tile([128, TOK_SUPER], mybir.dt.float32)
                    for kt in range(KT):
                        wv = wsb[kt // KPC][:, kt % KPC, ts(fb, 128)]
                        nc.tensor.matmul(ps[:], wv, xslice(kt),
                                         start=(kt == 0), stop=(kt == KT - 1))
                    osb = opool.tile([128, TOK_SUPER], mybir.dt.bfloat16)
                    nc.vector.tensor_copy(osb[:], ps[:])
                    nc.scalar.dma_start(
                        out[ts(fb, 128), ts(st, TOK_SUPER)], osb[:])

            compute_supertile(
                0, lambda kt: x0[kt // KPC][:, kt % KPC, :])
            for st in range(1, N_SUPER):
                xs = xpool.tile([128, KT, TOK_SUPER], mybir.dt.bfloat16,
                                tag="xs")
                nc.sync.dma_start(xs[:], xh[st])
                compute_supertile(st, lambda kt, xs=xs: xs[:, kt, :])
    nc.compile()
    return nc


def _host_inputs(x, W_qkv):
    bf16 = ml_dtypes.bfloat16
    xf = np.asarray(x, dtype=np.float32).reshape(M, E).astype(bf16)
    # xh[st, p, kt, m] = x[st*512+m, kt*128+p]
    xh = np.ascontiguousarray(
        xf.reshape(N_SUPER, TOK_SUPER, KT, 128).transpose(0, 3, 2, 1))
    W = np.asarray(W_qkv, dtype=np.float32)
    in_maps = []
    for c in range(NCORES):
        rows = np.concatenate([W[o + 256 * c: o + 256 * c + 256]
                               for o in (0, E, 2 * E)])
        wt_c = np.ascontiguousarray(
            rows.reshape(FPC, KT, 128).astype(bf16).transpose(2, 1, 0))
        in_maps.append({"xh": xh, "wt": wt_c})
    return in_maps


def kernel(x, W_qkv):
    from concourse.bass_utils import run_bass_kernel_spmd

    if "nc" not in _cache:
        _cache["nc"] = _build_program()
    nc = _cache["nc"]

    in_maps = _host_inputs(x, W_qkv)
    res = run_bass_kernel_spmd(nc, in_maps, core_ids=list(range(NCORES)))
    kernel._last_results = res

    q = np.empty((B, H, S, DK), np.float32)
    k = np.empty_like(q)
    v = np.empty_like(q)
    for c in range(NCORES):
        o = res.results[c]["out"]                       # [768, 16384] bf16
        # arr[b, fb, s, dk] = o[fb*128+dk, b*4096+s]
        arr = np.ascontiguousarray(
            o.reshape(FB, 128, B, S).transpose(2, 0, 3, 1)).astype(np.float32)
        for j in range(2):
            q[:, 2 * c + j] = arr[:, j]
            k[:, 2 * c + j] = arr[:, 2 + j]
            v[:, 2 * c + j] = arr[:, 4 + j]
    return q, k, v


# revision 11
# speedup vs baseline: 1.0161x; 1.0017x over previous
"""Fused QKV projection (nn.Linear premix) on 8 Trainium2 NeuronCores.

qkv = x @ W_qkv^T ; split into per-head q,k,v of shape [B,H,S,DK].

Sharding (tensor-parallel, per spec hint): the 3E=6144 output dim of
W_qkv is head-sharded across 8 cores.  Core c owns q-heads {2c,2c+1},
k-heads {2c,2c+1}, v-heads {2c,2c+1} -> 768 rows of W.  x is replicated.

Per-core GEMM: [16384 x 2048] @ [2048 x 768] in bf16 (TensorE peak
78.6 TF/s -> ~656 us floor).

Device kernel design (v3 — W-stationary, N=512):
  - W-stationary matmuls: lhsT = W^T block [128k, 128f], moving rhs =
    x^T [128k, 512 tokens] -> PSUM [128f, 512tok] fp32 = exactly one
    full PSUM bank.  3072 MMs total (vs 4096 for x-stationary N=384):
    same streamed columns, 25% fewer instruction overheads.
  - Host pre-layouts so every DMA line is long and contiguous:
      xh  [32, 128, 16, 512] : xh[st,p,kt,m] = x[st*512+m, kt*128+p]
          -> one 2 MB DMA per 512-token super-tile with 16 KB lines
             (supertile 0 split into 4x512 KB chunks, 4 KB lines).
      wt  [128, 16, 768]     : wt[p,kt,f] = W_c[f, kt*128+p]
  - Warm-up burst: ~11 N=512 matmuls on a zeroed SBUF tile right after
    the preamble barrier keep the PE busy through the initial HBM loads
    and flip the HAM clock gate to 8/8 before the first real matmul.
  - Head is data-bound on (W 3MB + x-st0 2MB): both go on the sync
    HWDGE ring, interleaved W-chunk/x-chunk, so W (which every chain
    needs in full) finishes as early as possible.
  - Output: per (supertile, f-block) chain, VectorE casts PSUM fp32 ->
    bf16 [128, 512] SBUF tile; one 128 KB DMA on the scalar ring writes
    out[f-block, token-slice] (1 KB rows).  In/out streams never share
    a descriptor ring.
"""

import numpy as np
import ml_dtypes

B, S, E, H, DK = 4, 4096, 2048, 16, 128
M = B * S              # 16384 tokens
NCORES = 8
FPC = 3 * E // NCORES  # 768 output features per core (6 head-slices)
FB = FPC // 128        # 6 feature blocks (head-slices)
KT = E // 128          # 16 contraction subtiles
TOK_SUPER = 512
N_SUPER = M // TOK_SUPER

_cache = {}


def _build_program():
    import concourse.bass as bass
    import concourse.bacc as bacc
    import concourse.mybir as mybir
    from concourse import tile

    ts = bass.ts
    nc = bacc.Bacc("TRN2", target_bir_lowering=False, debug=False,
                   num_devices=NCORES)
    xh = nc.dram_tensor("xh", [N_SUPER, 128, KT, TOK_SUPER],
                        mybir.dt.bfloat16, kind="ExternalInput")
    wt = nc.dram_tensor("wt", [FB, 128, KT, 128], mybir.dt.bfloat16,
                        kind="ExternalInput")
    out = nc.dram_tensor("out", [FPC, M], mybir.dt.bfloat16,
                         kind="ExternalOutput")

    KC = 8                 # kt chunks for W and for supertile 0's x
    KPC = KT // KC         # 2 kt per chunk
    with tile.TileContext(nc) as tc:
        with tc.tile_pool(name="zpool", bufs=1) as zpool, \
             tc.tile_pool(name="wpool", bufs=1) as wpool, \
             tc.tile_pool(name="x0pool", bufs=1) as x0pool, \
             tc.tile_pool(name="xpool", bufs=3) as xpool, \
             tc.tile_pool(name="opool", bufs=4) as opool, \
             tc.tile_pool(name="wmps", bufs=1, space="PSUM") as wmps, \
             tc.tile_pool(name="pspool", bufs=3, space="PSUM") as pspool:
            # ---- warm-up: PE busy from ~t0 so HAM un-throttles before
            # the first real matmul; zero source, dedicated PSUM bank.
            zt = zpool.tile([128, 512], mybir.dt.bfloat16, tag="z")
            nc.vector.memset(zt[:], 0.0)
            pw = wmps.tile([128, 512], mybir.dt.float32, tag="pw")
            for _ in range(8):
                nc.tensor.matmul(pw[:], zt[:, 0:128], zt[:],
                                 start=True, stop=True)

            # ---- W (fb-major, one 512 KB DMA per feature block) and
            # x-supertile-0 chunks interleaved on the sync ring.  Chain
            # fb only needs W[fb], so the x0 chunks get priority and
            # each W block arrives just-in-time for its chain.
            wsb = [None] * FB
            x0 = [None] * KC

            def load_w(fb):
                wc = wpool.tile([128, KT, 128], mybir.dt.bfloat16,
                                tag=f"w{fb}")
                nc.sync.dma_start(wc[:], wt[fb])
                wsb[fb] = wc

            def load_x0(kc):
                xc = x0pool.tile([128, KPC, TOK_SUPER], mybir.dt.bfloat16,
                                 tag=f"x0{kc}")
                nc.sync.dma_start(xc[:], xh[0, :, ts(kc, KPC), :])
                x0[kc] = xc

            load_x0(0); load_w(0)
            load_x0(1); load_x0(2); load_w(1)
            load_x0(3); load_x0(4); load_w(2)
            load_x0(5); load_x0(6); load_w(3)
            load_x0(7); load_w(4); load_w(5)

            def compute_supertile(st, xslice):
                # xslice(kt) -> [128, 512] moving operand (x^T)
                for fb in range(FB):
                    if st == N_SUPER - 1 and fb == FB - 1:
                        # final chain: two N=256 halves so the last
                        # PSUM drain + out-DMA receipt is half-size
                        # (shortens the serial kernel tail)
                        for h in range(2):
                            ps = pspool.tile([128, TOK_SUPER // 2],
                                             mybir.dt.float32)
                            for kt in range(KT):
                                wv = wsb[fb][:, kt, :]
                                nc.tensor.matmul(
                                    ps[:], wv,
                                    xslice(kt)[:, ts(h, TOK_SUPER // 2)],
                                    start=(kt == 0), stop=(kt == KT - 1))
                            osb = opool.tile([128, TOK_SUPER // 2],
                                             mybir.dt.bfloat16)
                            nc.vector.tensor_copy(osb[:], ps[:])
                            nc.scalar.dma_start(
                                out[ts(fb, 128),
                                    st * TOK_SUPER + h * (TOK_SUPER // 2):
                                    st * TOK_SUPER + (h + 1) * (TOK_SUPER // 2)],
                                osb[:])
                        continue
                    ps = pspool.tile([128, TOK_SUPER], mybir.dt.float32)
                    for kt in range(KT):
                        wv = wsb[fb][:, kt, :]
                        nc.tensor.matmul(ps[:], wv, xslice(kt),
                                         start=(kt == 0), stop=(kt == KT - 1))
                    osb = opool.tile([128, TOK_SUPER], mybir.dt.bfloat16)
                    nc.vector.tensor_copy(osb[:], ps[:])
                    nc.scalar.dma_start(
                        out[ts(fb, 128), ts(st, TOK_SUPER)], osb[:])

            compute_supertile(
                0, lambda kt: x0[kt // KPC][:, kt % KPC, :])
            for st in range(1, N_SUPER):
                xs = xpool.tile([128, KT, TOK_SUPER], mybir.dt.bfloat16,
                                tag="xs")
                nc.sync.dma_start(xs[:], xh[st])
                compute_supertile(st, lambda kt, xs=xs: xs[:, kt, :])
    nc.compile()
    return nc


def _host_inputs(x, W_qkv):
    bf16 = ml_dtypes.bfloat16
    xf = np.asarray(x, dtype=np.float32).reshape(M, E).astype(bf16)
    # xh[st, p, kt, m] = x[st*512+m, kt*128+p]
    xh = np.ascontiguousarray(
        xf.reshape(N_SUPER, TOK_SUPER, KT, 128).transpose(0, 3, 2, 1))
    W = np.asarray(W_qkv, dtype=np.float32)
    in_maps = []
    for c in range(NCORES):
        rows = np.concatenate([W[o + 256 * c: o + 256 * c + 256]
                               for o in (0, E, 2 * E)])
        # wt[fb, p, kt, f] = W_c[fb*128+f, kt*128+p]
        wt_c = np.ascontiguousarray(
            rows.reshape(FB, 128, KT, 128).astype(bf16).transpose(0, 3, 2, 1))
        in_maps.append({"xh": xh, "wt": wt_c})
    return in_maps


def kernel(x, W_qkv):
    from concourse.bass_utils import run_bass_kernel_spmd

    if "nc" not in _cache:
        _cache["nc"] = _build_program()
    nc = _cache["nc"]

    in_maps = _host_inputs(x, W_qkv)
    res = run_bass_kernel_spmd(nc, in_maps, core_ids=list(range(NCORES)))
    kernel._last_results = res

    q = np.empty((B, H, S, DK), np.float32)
    k = np.empty_like(q)
    v = np.empty_like(q)
    for c in range(NCORES):
        o = res.results[c]["out"]                       # [768, 16384] bf16
        # arr[b, fb, s, dk] = o[fb*128+dk, b*4096+s]
        arr = np.ascontiguousarray(
            o.reshape(FB, 128, B, S).transpose(2, 0, 3, 1)).astype(np.float32)
        for j in range(2):
            q[:, 2 * c + j] = arr[:, j]
            k[:, 2 * c + j] = arr[:, 2 + j]
            v[:, 2 * c + j] = arr[:, 4 + j]
    return q, k, v
